# revision 18
# baseline (speedup 1.0000x reference)
"""Trainium2 Bass kernel for ContextAttentionMaskLuong.

Reference computation (per batch b):
    keys  = x @ W                       [B,S,D]
    query = tanh(c @ Wc + b)            [B,D]
    eij   = scale * <query, keys_s>     [B,S]
    a     = exp(eij - max) * mask; a /= (sum(a) + 1e-7)
    out   = sum_s a[s] * x[s,:]         [B,D]

Key rewrite: eij[b,s] = <x[b,s,:], q2[b]> with q2[b] = scale * (W @ query[b]),
which removes the [B,S,D]x[D,D] matmul entirely. The kernel is then one
streaming pass over x (memory-bound).

Precision: the softmax is winner-take-most (sigma_eij ~ 20), so the whole
score path (x, W, Wc, c, query, q2, eij) must stay fp32 — bf16 anywhere in it
costs ~1.6e-2 global error. Only the final pooling (a^T x) runs bf16 on PE
(~2e-3), since fp32 PE matmul is a slow 2-pass emulation.

Engine split per core (2 batches):
  - PE: 64 fp32 Wc matmuls (single-shot) + tiny row-transposes + softmax
    cross-partition reductions + 64 bf16 pooling matmuls
  - DVE: q2 = W @ query as 16 fused mult+reduce over natural W chunks,
    eij as 32 fused mult+reduce over x tiles, small softmax ops
  - ACT: tanh/exp, psum->sbuf row copies (scale folded in)
  - POOL(gpsimd): mask cast DMA, partition broadcasts, x bf16 casts
  - DMA order on the SP FIFO: consts, Wc, W, x (largest last)
"""

import numpy as np

B, S, D = 16, 2048, 1024
NCORES = 8
BPC = B // NCORES  # batches per core
EPS = 1e-7

TS = 4  # x tiles per batch
QT = 4  # s-rows per partition per tile
XF = QT * D  # x tile free size (4096)
SBLK = S // TS  # s-block per tile (512)
KD = D // 128  # 8 chunks of 128 along d/e/c

_CACHE = {}


def _build():
    import concourse.bass as bass
    import concourse.mybir as mybir
    import concourse.tile as tile
    from concourse import bacc
    from concourse.masks import make_identity

    fp32 = mybir.dt.float32
    bf16 = mybir.dt.bfloat16
    i32 = mybir.dt.int32
    AF = mybir.ActivationFunctionType
    OP = mybir.AluOpType
    ts = bass.ts

    nc = bacc.Bacc(None)

    x_d = nc.dram_tensor("x", [BPC, S, D], fp32, kind="ExternalInput")
    mask_d = nc.dram_tensor("mask", [BPC, S], i32, kind="ExternalInput")
    c_d = nc.dram_tensor("c", [BPC, D], fp32, kind="ExternalInput")
    w_d = nc.dram_tensor("W", [D, D], fp32, kind="ExternalInput")
    wc_d = nc.dram_tensor("Wc", [D, D], fp32, kind="ExternalInput")
    b_d = nc.dram_tensor("b", [D], fp32, kind="ExternalInput")
    scale_d = nc.dram_tensor("scale", [1], fp32, kind="ExternalInput")
    out_d = nc.dram_tensor("out", [BPC, D], fp32, kind="ExternalOutput")

    with tile.TileContext(nc) as tc:
        with (
            tc.tile_pool(name="const", bufs=1) as const,
            tc.tile_pool(name="xp", bufs=5) as xp,
            tc.tile_pool(name="xbf", bufs=2 * TS) as xbp,
            tc.tile_pool(name="wst", bufs=3) as wst,
            tc.tile_pool(name="persist", bufs=1) as persist,
            tc.tile_pool(name="scratch", bufs=1) as scratch,
            tc.tile_pool(name="psum", bufs=1, space="PSUM") as pp,
        ):
            # ---------- constants / small loads ----------
            identity32 = const.tile([128, 128], fp32, tag="identity32")
            make_identity(nc, identity32)
            ones1 = const.tile([1, 128], fp32, tag="ones1")
            nc.vector.memset(ones1, 1.0)
            ones_col = const.tile([128, 1], fp32, tag="ones_col")
            nc.vector.memset(ones_col, 1.0)

            scale_sb = const.tile([1, 1], fp32, tag="scale")
            nc.sync.dma_start(out=scale_sb, in_=scale_d[None, :])

            # bias with e on partitions: biasT[p, k] = b[128k+p]
            biasT = const.tile([128, KD], fp32, tag="biasT")
            nc.sync.dma_start(out=biasT, in_=b_d.rearrange("(k p) -> p k", p=128))

            # c transposed: cT[p, b, k] = c[b, 128k+p]
            cT = const.tile([128, BPC, KD], fp32, tag="cT")
            nc.sync.dma_start(out=cT, in_=c_d.rearrange("b (k p) -> p b k", p=128))

            # mask (cast int32 -> f32 during DMA), layout matches eij
            mask_f = []
            for b in range(BPC):
                mf = persist.tile([128, TS, QT], fp32, tag=f"mask{b}")
                nc.gpsimd.dma_start(
                    out=mf,
                    in_=mask_d[b].rearrange("(t p q) -> p t q", p=128, q=QT),
                )
                mask_f.append(mf)

            # ---------- Wc phase: queryT[e-part, ke, b]  (fp32 PE) ----------
            q_acc = const.tile([128, KD, BPC], fp32, tag="q_acc")
            for kc in range(KD):
                wc_t = wst.tile([128, D], fp32, tag="wstream", name="wc_t")
                nc.sync.dma_start(out=wc_t, in_=wc_d[ts(kc, 128), :])
                psq = pp.tile([128, KD, BPC], fp32, tag="psq", bufs=2, name="psq")
                for ke in range(KD):
                    nc.tensor.matmul(
                        psq[:, ke, :],
                        wc_t[:, ts(ke, 128)],
                        cT[:, :, kc],
                        start=True,
                        stop=True,
                    )
                if kc == 0:
                    nc.vector.tensor_copy(q_acc, psq)
                else:
                    nc.vector.tensor_tensor(q_acc, q_acc, psq, op=OP.add)
            q_biased = const.tile([128, KD, BPC], fp32, tag="q_biased")
            for b in range(BPC):
                nc.vector.tensor_tensor(
                    q_biased[:, :, b], q_acc[:, :, b], biasT, op=OP.add
                )
            queryT = const.tile([128, KD, BPC], fp32, tag="queryT")
            nc.scalar.activation(queryT, q_biased, AF.Tanh)

            # query rows: per (b, ke) transpose [128,1] -> [1,128] on PE,
            # gather into one [1, D] row, broadcast to 128 partitions
            qb128 = []
            for b in range(BPC):
                qtr = pp.tile(
                    [1, KD, 128], fp32, tag="rowps", bufs=2, name="qtr"
                )
                for ke in range(KD):
                    nc.tensor.transpose(
                        qtr[:, ke, :], queryT[:, ke, b : b + 1], identity32
                    )
                qrow = persist.tile([1, D], fp32, tag=f"qrow{b}")
                nc.scalar.copy(qrow, qtr)
                qb = persist.tile([128, D], fp32, tag=f"qb128_{b}")
                nc.gpsimd.partition_broadcast(qb, qrow)
                qb128.append(qb)

            # ---------- W phase: q2 = scale*(W @ query) on DVE ----------
            # q2col[p, kd] = sum_e W[128*kd+p, e] * query[e]
            q2col = [
                persist.tile([128, KD], fp32, tag=f"q2col{b}", name=f"q2col{b}")
                for b in range(BPC)
            ]
            for kd in range(KD):
                wn = wst.tile([128, D], fp32, tag="wstream", name="wn")
                nc.sync.dma_start(out=wn, in_=w_d[ts(kd, 128), :])
                for b in range(BPC):
                    wsc = scratch.tile([128, D], fp32, tag="ttr_out", name="wsc")
                    nc.vector.scalar_tensor_tensor(
                        out=wsc,
                        in0=wn,
                        scalar=1.0,
                        in1=qb128[b],
                        op0=OP.mult,
                        op1=OP.mult,
                        accum_out=q2col[b][:, kd : kd + 1],
                    )

            # q2 rows: transpose each column chunk, gather, scale, broadcast
            q2b = []
            for b in range(BPC):
                q2tr = pp.tile(
                    [1, KD, 128], fp32, tag="rowps", bufs=2, name="q2tr"
                )
                for kd in range(KD):
                    nc.tensor.transpose(
                        q2tr[:, kd, :], q2col[b][:, kd : kd + 1], identity32
                    )
                q2row = persist.tile([1, D], fp32, tag=f"q2row{b}")
                nc.scalar.mul(q2row, q2tr, scale_sb)
                qb = persist.tile([128, D], fp32, tag=f"qb128_{b}", name="q2bb")
                nc.gpsimd.partition_broadcast(qb, q2row)
                q2b.append(qb)

            # ---------- x DMAs (fp32) + bf16 pooling copies on POOL --------
            x_tiles = [[None] * TS for _ in range(BPC)]
            xbf_tiles = [[None] * TS for _ in range(BPC)]
            for b in range(BPC):
                for t in range(TS):
                    xt = xp.tile([128, XF], fp32, tag="xt")
                    nc.sync.dma_start(
                        out=xt,
                        in_=x_d[b, ts(t, SBLK), :].rearrange(
                            "(p q) d -> p (q d)", p=128
                        ),
                    )
                    x_tiles[b][t] = xt
                    xb = xbp.tile([128, XF], bf16, tag="xbf")
                    nc.gpsimd.tensor_copy(xb, xt)
                    xbf_tiles[b][t] = xb

            outrow = const.tile([1, BPC * D], fp32, tag="outrow")

            # ---------- main pass ----------
            for b in range(BPC):
                # eij[p, t, q] = <x[s], q2[b]>  for s = 512t + 4p + q
                eij = persist.tile([128, TS, QT], fp32, tag=f"eij{b}")
                for t in range(TS):
                    for q in range(QT):
                        sc = scratch.tile([128, D], fp32, tag="ttr_out")
                        nc.vector.scalar_tensor_tensor(
                            out=sc,
                            in0=x_tiles[b][t][:, ts(q, D)],
                            scalar=1.0,
                            in1=q2b[b],
                            op0=OP.mult,
                            op1=OP.mult,
                            accum_out=eij[:, t, q : q + 1],
                        )

                # masked softmax (unnormalized; normalization folded into out)
                m1 = scratch.tile([128, 1], fp32, tag="m1")
                nc.vector.reduce_max(m1, eij, axis=mybir.AxisListType.XY)
                pmax = pp.tile([1, 128], fp32, tag="pb", bufs=2, name="pmax")
                nc.tensor.transpose(pmax, m1, identity32)
                negmx = scratch.tile([1, 1], fp32, tag="negmx")
                nc.vector.reduce_max(
                    negmx, pmax, axis=mybir.AxisListType.X, negate=True
                )
                pbm = pp.tile([128, 1], fp32, tag="pb", bufs=2, name="pbm")
                nc.tensor.matmul(pbm, ones1, negmx, start=True, stop=True)
                negm = scratch.tile([128, 1], fp32, tag="negm")
                nc.scalar.copy(negm, pbm)
                a_b = persist.tile([128, TS, QT], fp32, tag=f"a{b}")
                nc.scalar.activation(a_b, eij, AF.Exp, bias=negm, scale=1.0)
                nc.vector.tensor_tensor(a_b, a_b, mask_f[b], op=OP.mult)

                s1 = scratch.tile([128, 1], fp32, tag="s1")
                nc.vector.reduce_sum(s1, a_b, axis=mybir.AxisListType.XY)
                ssum = pp.tile([1, 1], fp32, tag="pb", bufs=2, name="ssum")
                nc.tensor.matmul(ssum, s1, ones_col, start=True, stop=True)
                den = scratch.tile([1, 1], fp32, tag="den")
                nc.vector.tensor_scalar_add(den, ssum, EPS)
                rden = scratch.tile([1, 1], fp32, tag="rden")
                nc.vector.reciprocal(rden, den)

                # bf16 copy of the softmax weights for the PE pooling
                a_bf = persist.tile([128, TS, QT], bf16, tag=f"abf{b}")
                nc.scalar.copy(a_bf, a_b)

                # out[b, d] = rden * sum_s a[s] x[s, d]   (bf16 PE pooling)
                for h in range(2):
                    po = pp.tile([1, 512], fp32, tag="pb", bufs=2, name="po")
                    n = 0
                    for t in range(TS):
                        for q in range(QT):
                            nc.tensor.matmul(
                                po,
                                a_bf[:, t, q : q + 1],
                                xbf_tiles[b][t][
                                    :, q * D + h * 512 : q * D + (h + 1) * 512
                                ],
                                start=(n == 0),
                                stop=(n == TS * QT - 1),
                            )
                            n += 1
                    nc.vector.tensor_scalar_mul(
                        outrow[:, b * D + h * 512 : b * D + (h + 1) * 512],
                        po,
                        rden,
                    )

            nc.sync.dma_start(
                out=out_d.rearrange("b d -> (b d)")[None, :], in_=outrow
            )

    nc.compile()
    return nc


def _get_nc():
    if "nc" not in _CACHE:
        _CACHE["nc"] = _build()
    return _CACHE["nc"]


def run(inputs, trace=False):
    from concourse.bass_utils import run_bass_kernel_spmd

    x = np.ascontiguousarray(inputs["x"], dtype=np.float32)
    mask = np.ascontiguousarray(inputs["mask"], dtype=np.int32)
    c = np.ascontiguousarray(inputs["c"], dtype=np.float32)
    W = np.ascontiguousarray(inputs["W"], dtype=np.float32)
    Wc = np.ascontiguousarray(inputs["Wc"], dtype=np.float32)
    b = np.ascontiguousarray(inputs["b"], dtype=np.float32)
    scale = np.ascontiguousarray(inputs["scale"], dtype=np.float32)

    in_maps = []
    for i in range(NCORES):
        sl = slice(i * BPC, (i + 1) * BPC)
        in_maps.append(
            {
                "x": x[sl],
                "mask": mask[sl],
                "c": c[sl],
                "W": W,
                "Wc": Wc,
                "b": b,
                "scale": scale,
            }
        )

    nc = _get_nc()
    res = run_bass_kernel_spmd(
        nc, in_maps, core_ids=list(range(NCORES)), trace=trace
    )
    out = np.concatenate([res.results[i]["out"] for i in range(NCORES)], axis=0)
    return out.astype(np.float32), res


def kernel(**inputs):
    out, _ = run(inputs, trace=False)
    return out


# revision 20
# speedup vs baseline: 1.9024x; 1.9024x over previous
"""Trainium2 Bass kernel for ContextAttentionMaskLuong.

Reference computation (per batch b):
    keys  = x @ W                       [B,S,D]
    query = tanh(c @ Wc + b)            [B,D]
    eij   = scale * <query, keys_s>     [B,S]
    a     = exp(eij - max) * mask; a /= (sum(a) + 1e-7)
    out   = sum_s a[s] * x[s,:]         [B,D]

Key rewrite: eij[b,s] = <x[b,s,:], q2[b]> with q2[b] = scale * (W @ query[b]),
which removes the [B,S,D]x[D,D] matmul entirely. The kernel is then one
streaming pass over x (memory-bound).

Precision: the softmax is winner-take-most (sigma_eij ~ 20), so the whole
score path (x, W, Wc, c, query, q2, eij) must stay fp32 — bf16 anywhere in it
costs ~1.6e-2 global error. Only the final pooling (a^T x) runs bf16 on PE
(~2e-3), since fp32 PE matmul is a slow 2-pass emulation.

Engine split per core (2 batches):
  - PE: 16 fp32 Wc matmuls (cT stationary, Wc moving, PSUM kc-accumulate;
    query comes out as rows) + q2-row transposes + softmax reductions +
    64 bf16 pooling matmuls
  - DVE: q2 = W @ query as 16 fused mult+reduce over natural W chunks,
    eij as 32 fused mult+reduce over x tiles, small softmax ops
  - ACT: tanh/exp, x bf16 pooling casts (POOL casts lock DVE out of the
    shared SBUF port), psum->sbuf row copies (scale folded in)
  - POOL(gpsimd): mask cast DMA, partition broadcasts only
  - DMA order on the SP FIFO: consts, Wc, W, x (largest last)
"""

import numpy as np

B, S, D = 16, 2048, 1024
NCORES = 8
BPC = B // NCORES  # batches per core
EPS = 1e-7

TS = 4  # x tiles per batch
QT = 4  # s-rows per partition per tile
XF = QT * D  # x tile free size (4096)
SBLK = S // TS  # s-block per tile (512)
KD = D // 128  # 8 chunks of 128 along d/e/c

_CACHE = {}


def _build():
    import concourse.bass as bass
    import concourse.mybir as mybir
    import concourse.tile as tile
    from concourse import bacc
    from concourse.masks import make_identity

    fp32 = mybir.dt.float32
    bf16 = mybir.dt.bfloat16
    i32 = mybir.dt.int32
    AF = mybir.ActivationFunctionType
    OP = mybir.AluOpType
    ts = bass.ts

    nc = bacc.Bacc(None)

    x_d = nc.dram_tensor("x", [BPC, S, D], fp32, kind="ExternalInput")
    mask_d = nc.dram_tensor("mask", [BPC, S], i32, kind="ExternalInput")
    c_d = nc.dram_tensor("c", [BPC, D], fp32, kind="ExternalInput")
    w_d = nc.dram_tensor("W", [D, D], fp32, kind="ExternalInput")
    wc_d = nc.dram_tensor("Wc", [D, D], fp32, kind="ExternalInput")
    b_d = nc.dram_tensor("b", [D], fp32, kind="ExternalInput")
    scale_d = nc.dram_tensor("scale", [1], fp32, kind="ExternalInput")
    out_d = nc.dram_tensor("out", [BPC, D], fp32, kind="ExternalOutput")

    with tile.TileContext(nc) as tc:
        with (
            tc.tile_pool(name="const", bufs=1) as const,
            tc.tile_pool(name="xp", bufs=5) as xp,
            tc.tile_pool(name="xbf", bufs=2 * TS) as xbp,
            tc.tile_pool(name="wst", bufs=4) as wst,
            tc.tile_pool(name="persist", bufs=1) as persist,
            tc.tile_pool(name="scratch", bufs=1) as scratch,
            tc.tile_pool(name="psum", bufs=1, space="PSUM") as pp,
        ):
            # ---------- constants / small loads ----------
            identity32 = const.tile([128, 128], fp32, tag="identity32")
            make_identity(nc, identity32)
            ones1 = const.tile([1, 128], fp32, tag="ones1")
            nc.vector.memset(ones1, 1.0)
            ones_col = const.tile([128, 1], fp32, tag="ones_col")
            nc.vector.memset(ones_col, 1.0)

            scale_sb = const.tile([1, 1], fp32, tag="scale")
            nc.sync.dma_start(out=scale_sb, in_=scale_d[None, :])

            # bias rows, replicated to both batch partitions
            bias2 = const.tile([BPC, D], fp32, tag="bias2")
            for b in range(BPC):
                nc.sync.dma_start(out=bias2[b : b + 1, :], in_=b_d[None, :])

            # c transposed: cT[p, b, k] = c[b, 128k+p]
            cT = const.tile([128, BPC, KD], fp32, tag="cT")
            nc.sync.dma_start(out=cT, in_=c_d.rearrange("b (k p) -> p b k", p=128))

            # mask (cast int32 -> f32 during DMA), layout matches eij
            mask_f = []
            for b in range(BPC):
                mf = persist.tile([128, TS, QT], fp32, tag=f"mask{b}")
                nc.gpsimd.dma_start(
                    out=mf,
                    in_=mask_d[b].rearrange("(t p q) -> p t q", p=128, q=QT),
                )
                mask_f.append(mf)

            # ---------- Wc phase: query rows [2, D] (fp32 PE) ----------
            # swap roles: cT chunk is the stationary operand (2 columns),
            # Wc streams as the moving operand; accumulate over kc in PSUM
            # (two banks, one accumulation group per 512-column half).
            qpr = pp.tile([BPC, D], fp32, tag="qpr", name="qpr")
            for kc in range(KD):
                wc_t = wst.tile([128, D], fp32, tag="wstream", name="wc_t")
                nc.sync.dma_start(out=wc_t, in_=wc_d[ts(kc, 128), :])
                for h in range(2):
                    nc.tensor.matmul(
                        qpr[:, ts(h, 512)],
                        cT[:, :, kc],
                        wc_t[:, ts(h, 512)],
                        start=(kc == 0),
                        stop=(kc == KD - 1),
                    )
            rows2 = const.tile([BPC, D], fp32, tag="rows2")
            nc.vector.tensor_tensor(rows2, qpr, bias2, op=OP.add)
            rows2t = const.tile([BPC, D], fp32, tag="rows2t")
            nc.scalar.activation(rows2t, rows2, AF.Tanh)

            # broadcast each query row to all 128 partitions (partition 1 is
            # unreachable for engines: bounce row 1 down via SBUF->SBUF DMA)
            qrow1 = persist.tile([1, D], fp32, tag="qrow1")
            nc.sync.dma_start(out=qrow1, in_=rows2t[1:2, :])
            qb128 = []
            for b in range(BPC):
                qb = persist.tile([128, D], fp32, tag=f"qb128_{b}")
                nc.gpsimd.partition_broadcast(
                    qb, rows2t[0:1, :] if b == 0 else qrow1
                )
                qb128.append(qb)

            # ---------- W phase: q2 = scale*(W @ query) on DVE ----------
            # q2col[p, kd] = sum_e W[128*kd+p, e] * query[e]
            q2col = [
                persist.tile([128, KD], fp32, tag=f"q2col{b}", name=f"q2col{b}")
                for b in range(BPC)
            ]
            for kd in range(KD):
                wn = wst.tile([128, D], fp32, tag="wstream", name="wn")
                nc.sync.dma_start(out=wn, in_=w_d[ts(kd, 128), :])
                for b in range(BPC):
                    wsc = scratch.tile([128, D], fp32, tag="ttr_out", name="wsc")
                    nc.vector.scalar_tensor_tensor(
                        out=wsc,
                        in0=wn,
                        scalar=1.0,
                        in1=qb128[b],
                        op0=OP.mult,
                        op1=OP.mult,
                        accum_out=q2col[b][:, kd : kd + 1],
                    )

            # q2 rows: transpose each column chunk, gather, scale, broadcast
            q2b = []
            for b in range(BPC):
                q2tr = pp.tile(
                    [1, KD, 128], fp32, tag="rowps", bufs=2, name="q2tr"
                )
                for kd in range(KD):
                    nc.tensor.transpose(
                        q2tr[:, kd, :], q2col[b][:, kd : kd + 1], identity32
                    )
                q2row = persist.tile([1, D], fp32, tag=f"q2row{b}")
                nc.scalar.mul(q2row, q2tr, scale_sb)
                qb = persist.tile([128, D], fp32, tag=f"qb128_{b}", name="q2bb")
                nc.gpsimd.partition_broadcast(qb, q2row)
                q2b.append(qb)

            # ---------- x DMAs (fp32) + bf16 pooling copies on POOL --------
            x_tiles = [[None] * TS for _ in range(BPC)]
            xbf_tiles = [[None] * TS for _ in range(BPC)]
            for b in range(BPC):
                for t in range(TS):
                    xt = xp.tile([128, XF], fp32, tag="xt")
                    nc.sync.dma_start(
                        out=xt,
                        in_=x_d[b, ts(t, SBLK), :].rearrange(
                            "(p q) d -> p (q d)", p=128
                        ),
                    )
                    x_tiles[b][t] = xt
                    xb = xbp.tile([128, XF], bf16, tag="xbf")
                    nc.scalar.copy(xb, xt)
                    xbf_tiles[b][t] = xb

            outrow = const.tile([1, BPC * D], fp32, tag="outrow")

            # ---------- main pass ----------
            for b in range(BPC):
                # eij[p, t, q] = <x[s], q2[b]>  for s = 512t + 4p + q
                eij = persist.tile([128, TS, QT], fp32, tag=f"eij{b}")
                for t in range(TS):
                    for q in range(QT):
                        sc = scratch.tile([128, D], fp32, tag="ttr_out")
                        nc.vector.scalar_tensor_tensor(
                            out=sc,
                            in0=x_tiles[b][t][:, ts(q, D)],
                            scalar=1.0,
                            in1=q2b[b],
                            op0=OP.mult,
                            op1=OP.mult,
                            accum_out=eij[:, t, q : q + 1],
                        )

                # masked softmax (unnormalized; normalization folded into out)
                m1 = scratch.tile([128, 1], fp32, tag="m1")
                nc.vector.reduce_max(m1, eij, axis=mybir.AxisListType.XY)
                pmax = pp.tile([1, 128], fp32, tag="pb", bufs=2, name="pmax")
                nc.tensor.transpose(pmax, m1, identity32)
                negmx = scratch.tile([1, 1], fp32, tag="negmx")
                nc.vector.reduce_max(
                    negmx, pmax, axis=mybir.AxisListType.X, negate=True
                )
                pbm = pp.tile([128, 1], fp32, tag="pb", bufs=2, name="pbm")
                nc.tensor.matmul(pbm, ones1, negmx, start=True, stop=True)
                negm = scratch.tile([128, 1], fp32, tag="negm")
                nc.scalar.copy(negm, pbm)
                a_b = persist.tile([128, TS, QT], fp32, tag=f"a{b}")
                nc.scalar.activation(a_b, eij, AF.Exp, bias=negm, scale=1.0)
                nc.vector.tensor_tensor(a_b, a_b, mask_f[b], op=OP.mult)

                s1 = scratch.tile([128, 1], fp32, tag="s1")
                nc.vector.reduce_sum(s1, a_b, axis=mybir.AxisListType.XY)
                ssum = pp.tile([1, 1], fp32, tag="pb", bufs=2, name="ssum")
                nc.tensor.matmul(ssum, s1, ones_col, start=True, stop=True)
                den = scratch.tile([1, 1], fp32, tag="den")
                nc.vector.tensor_scalar_add(den, ssum, EPS)
                rden = scratch.tile([1, 1], fp32, tag="rden")
                nc.vector.reciprocal(rden, den)

                # bf16 copy of the softmax weights for the PE pooling
                a_bf = persist.tile([128, TS, QT], bf16, tag=f"abf{b}")
                nc.scalar.copy(a_bf, a_b)

                # out[b, d] = rden * sum_s a[s] x[s, d]   (bf16 PE pooling)
                for h in range(2):
                    po = pp.tile([1, 512], fp32, tag="pb", bufs=2, name="po")
                    n = 0
                    for t in range(TS):
                        for q in range(QT):
                            nc.tensor.matmul(
                                po,
                                a_bf[:, t, q : q + 1],
                                xbf_tiles[b][t][
                                    :, q * D + h * 512 : q * D + (h + 1) * 512
                                ],
                                start=(n == 0),
                                stop=(n == TS * QT - 1),
                            )
                            n += 1
                    nc.vector.tensor_scalar_mul(
                        outrow[:, b * D + h * 512 : b * D + (h + 1) * 512],
                        po,
                        rden,
                    )

            nc.sync.dma_start(
                out=out_d.rearrange("b d -> (b d)")[None, :], in_=outrow
            )

    nc.compile()
    return nc


def _get_nc():
    if "nc" not in _CACHE:
        _CACHE["nc"] = _build()
    return _CACHE["nc"]


def run(inputs, trace=False):
    from concourse.bass_utils import run_bass_kernel_spmd

    x = np.ascontiguousarray(inputs["x"], dtype=np.float32)
    mask = np.ascontiguousarray(inputs["mask"], dtype=np.int32)
    c = np.ascontiguousarray(inputs["c"], dtype=np.float32)
    W = np.ascontiguousarray(inputs["W"], dtype=np.float32)
    Wc = np.ascontiguousarray(inputs["Wc"], dtype=np.float32)
    b = np.ascontiguousarray(inputs["b"], dtype=np.float32)
    scale = np.ascontiguousarray(inputs["scale"], dtype=np.float32)

    in_maps = []
    for i in range(NCORES):
        sl = slice(i * BPC, (i + 1) * BPC)
        in_maps.append(
            {
                "x": x[sl],
                "mask": mask[sl],
                "c": c[sl],
                "W": W,
                "Wc": Wc,
                "b": b,
                "scale": scale,
            }
        )

    nc = _get_nc()
    res = run_bass_kernel_spmd(
        nc, in_maps, core_ids=list(range(NCORES)), trace=trace
    )
    out = np.concatenate([res.results[i]["out"] for i in range(NCORES)], axis=0)
    return out.astype(np.float32), res


def kernel(**inputs):
    out, _ = run(inputs, trace=False)
    return out


# revision 21
# speedup vs baseline: 3.2813x; 1.7248x over previous
"""Trainium2 Bass kernel for ContextAttentionMaskLuong.

Reference computation (per batch b):
    keys  = x @ W                       [B,S,D]
    query = tanh(c @ Wc + b)            [B,D]
    eij   = scale * <query, keys_s>     [B,S]
    a     = exp(eij - max) * mask; a /= (sum(a) + 1e-7)
    out   = sum_s a[s] * x[s,:]         [B,D]

Key rewrite: eij[b,s] = <x[b,s,:], q2[b]> with q2[b] = scale * (W @ query[b]),
which removes the [B,S,D]x[D,D] matmul entirely. The kernel is then one
streaming pass over x (memory-bound).

Precision: the softmax is winner-take-most (sigma_eij ~ 20), so the whole
score path (x, W, Wc, c, query, q2, eij) must stay fp32 — bf16 anywhere in it
costs ~1.6e-2 global error. Only the final pooling (a^T x) runs bf16 on PE
(~2e-3), since fp32 PE matmul is a slow 2-pass emulation.

Engine split per core (2 batches):
  - PE: 16 fp32 Wc matmuls (cT stationary, Wc moving, PSUM kc-accumulate;
    query comes out as rows) + q2-row transposes + softmax reductions +
    64 bf16 pooling matmuls
  - DVE: q2 = W @ query as 16 fused mult+reduce over natural W chunks,
    eij as 32 fused mult+reduce over x tiles, small softmax ops
  - ACT: tanh/exp, x bf16 pooling casts (POOL casts lock DVE out of the
    shared SBUF port), psum->sbuf row copies (scale folded in)
  - POOL(gpsimd): mask cast DMA, partition broadcasts only
  - DMA order on the SP FIFO: consts, Wc, W, x (largest last)
"""

import numpy as np

B, S, D = 16, 2048, 1024
NCORES = 8
BPC = B // NCORES  # batches per core
EPS = 1e-7

TS = 4  # x tiles per batch
QT = 4  # s-rows per partition per tile
XF = QT * D  # x tile free size (4096)
SBLK = S // TS  # s-block per tile (512)
KD = D // 128  # 8 chunks of 128 along d/e/c

_CACHE = {}


def _build():
    import concourse.bass as bass
    import concourse.mybir as mybir
    import concourse.tile as tile
    from concourse import bacc
    from concourse.masks import make_identity

    fp32 = mybir.dt.float32
    bf16 = mybir.dt.bfloat16
    i32 = mybir.dt.int32
    AF = mybir.ActivationFunctionType
    OP = mybir.AluOpType
    ts = bass.ts

    nc = bacc.Bacc(None)

    x_d = nc.dram_tensor("x", [BPC, S, D], fp32, kind="ExternalInput")
    mask_d = nc.dram_tensor("mask", [BPC, S], i32, kind="ExternalInput")
    c_d = nc.dram_tensor("c", [BPC, D], fp32, kind="ExternalInput")
    w_d = nc.dram_tensor("W", [D, D], fp32, kind="ExternalInput")
    wc_d = nc.dram_tensor("Wc", [D, D], fp32, kind="ExternalInput")
    b_d = nc.dram_tensor("b", [D], fp32, kind="ExternalInput")
    scale_d = nc.dram_tensor("scale", [1], fp32, kind="ExternalInput")
    out_d = nc.dram_tensor("out", [BPC, D], fp32, kind="ExternalOutput")

    with tile.TileContext(nc) as tc:
        with (
            tc.tile_pool(name="const", bufs=1) as const,
            tc.tile_pool(name="xp", bufs=5) as xp,
            tc.tile_pool(name="xbf", bufs=2 * TS) as xbp,
            tc.tile_pool(name="wst", bufs=7) as wst,
            tc.tile_pool(name="persist", bufs=1) as persist,
            tc.tile_pool(name="scratch", bufs=1) as scratch,
            tc.tile_pool(name="psum", bufs=1, space="PSUM") as pp,
        ):
            # ---------- constants / small loads ----------
            identity32 = const.tile([128, 128], fp32, tag="identity32")
            make_identity(nc, identity32)
            ones1 = const.tile([1, 128], fp32, tag="ones1")
            nc.vector.memset(ones1, 1.0)
            ones_col = const.tile([128, 1], fp32, tag="ones_col")
            nc.vector.memset(ones_col, 1.0)

            scale_sb = const.tile([1, 1], fp32, tag="scale")
            nc.sync.dma_start(out=scale_sb, in_=scale_d[None, :])

            # bias rows, replicated to both batch partitions
            bias2 = const.tile([BPC, D], fp32, tag="bias2")
            for b in range(BPC):
                nc.sync.dma_start(out=bias2[b : b + 1, :], in_=b_d[None, :])

            # c transposed: cT[p, b, k] = c[b, 128k+p]
            cT = const.tile([128, BPC, KD], fp32, tag="cT")
            nc.sync.dma_start(out=cT, in_=c_d.rearrange("b (k p) -> p b k", p=128))

            # mask (cast int32 -> f32 during DMA), layout matches eij
            mask_f = []
            for b in range(BPC):
                mf = persist.tile([128, TS, QT], fp32, tag=f"mask{b}")
                nc.gpsimd.dma_start(
                    out=mf,
                    in_=mask_d[b].rearrange("(t p q) -> p t q", p=128, q=QT),
                )
                mask_f.append(mf)

            # ---------- Wc phase: query rows [2, D] (fp32 PE) ----------
            # swap roles: cT chunk is the stationary operand (2 columns),
            # Wc streams as the moving operand; accumulate over kc in PSUM
            # (two banks, one accumulation group per 512-column half).
            qpr = pp.tile([BPC, D], fp32, tag="qpr", name="qpr")
            for kc in range(KD):
                wc_t = wst.tile([128, D], fp32, tag="wstream", name="wc_t")
                nc.sync.dma_start(out=wc_t, in_=wc_d[ts(kc, 128), :])
                for h in range(2):
                    nc.tensor.matmul(
                        qpr[:, ts(h, 512)],
                        cT[:, :, kc],
                        wc_t[:, ts(h, 512)],
                        start=(kc == 0),
                        stop=(kc == KD - 1),
                    )
            rows2 = const.tile([BPC, D], fp32, tag="rows2")
            nc.vector.tensor_tensor(rows2, qpr, bias2, op=OP.add)
            nc.scalar.activation(rows2, rows2, AF.Tanh)

            # broadcast each query row to all 128 partitions (partition 1 is
            # unreachable for engines: bounce row 1 down via SBUF->SBUF DMA)
            qrow1 = persist.tile([1, D], fp32, tag="rowx", bufs=2, name="qrow1")
            nc.sync.dma_start(out=qrow1, in_=rows2[1:2, :])
            qb128 = []
            for b in range(BPC):
                qb = persist.tile([128, D], fp32, tag=f"qb128_{b}")
                nc.gpsimd.partition_broadcast(
                    qb, rows2[0:1, :] if b == 0 else qrow1
                )
                qb128.append(qb)

            # ---------- W phase: q2 = scale*(W @ query) on DVE ----------
            # q2col[p, kd] = sum_e W[128*kd+p, e] * query[e]
            q2col = [
                persist.tile([128, KD], fp32, tag=f"q2col{b}", name=f"q2col{b}")
                for b in range(BPC)
            ]
            for kd in range(KD):
                wn = wst.tile([128, D], fp32, tag="wstream", name="wn")
                nc.sync.dma_start(out=wn, in_=w_d[ts(kd, 128), :])
                for b in range(BPC):
                    wsc = scratch.tile([128, D], fp32, tag="ttr_out", name="wsc")
                    nc.vector.scalar_tensor_tensor(
                        out=wsc,
                        in0=wn,
                        scalar=1.0,
                        in1=qb128[b],
                        op0=OP.mult,
                        op1=OP.mult,
                        accum_out=q2col[b][:, kd : kd + 1],
                    )

            # q2 rows: transpose each column chunk, gather, scale, broadcast
            q2b = []
            for b in range(BPC):
                q2tr = pp.tile(
                    [1, KD, 128], fp32, tag="rowps", bufs=2, name="q2tr"
                )
                for kd in range(KD):
                    nc.tensor.transpose(
                        q2tr[:, kd, :], q2col[b][:, kd : kd + 1], identity32
                    )
                q2row = persist.tile([1, D], fp32, tag="rowx", bufs=2, name="q2row")
                nc.scalar.mul(q2row, q2tr, scale_sb)
                qb = persist.tile([128, D], fp32, tag=f"qb128_{b}", name="q2bb")
                nc.gpsimd.partition_broadcast(qb, q2row)
                q2b.append(qb)

            # ---------- x DMAs (fp32) + bf16 pooling copies on POOL --------
            x_tiles = [[None] * TS for _ in range(BPC)]
            xbf_tiles = [[None] * TS for _ in range(BPC)]
            for b in range(BPC):
                for t in range(TS):
                    xt = xp.tile([128, XF], fp32, tag="xt")
                    nc.sync.dma_start(
                        out=xt,
                        in_=x_d[b, ts(t, SBLK), :].rearrange(
                            "(p q) d -> p (q d)", p=128
                        ),
                    )
                    x_tiles[b][t] = xt
                    xb = xbp.tile([128, XF], bf16, tag="xbf")
                    nc.scalar.copy(xb, xt)
                    xbf_tiles[b][t] = xb

            # ---------- main pass ----------
            out_rows = []
            for b in range(BPC):
                # eij[p, t, q] = <x[s], q2[b]>  for s = 512t + 4p + q
                eij = persist.tile([128, TS, QT], fp32, tag=f"eij{b}")
                for t in range(TS):
                    for q in range(QT):
                        sc = scratch.tile([128, D], fp32, tag="ttr_out")
                        nc.vector.scalar_tensor_tensor(
                            out=sc,
                            in0=x_tiles[b][t][:, ts(q, D)],
                            scalar=1.0,
                            in1=q2b[b],
                            op0=OP.mult,
                            op1=OP.mult,
                            accum_out=eij[:, t, q : q + 1],
                        )

                # masked softmax (unnormalized; normalization folded into out)
                m1 = scratch.tile([128, 1], fp32, tag="m1")
                nc.vector.reduce_max(m1, eij, axis=mybir.AxisListType.XY)
                pmax = pp.tile([1, 128], fp32, tag="pb", bufs=2, name="pmax")
                nc.tensor.transpose(pmax, m1, identity32)
                negmx = scratch.tile([1, 1], fp32, tag="negmx")
                nc.vector.reduce_max(
                    negmx, pmax, axis=mybir.AxisListType.X, negate=True
                )
                pbm = pp.tile([128, 1], fp32, tag="pb", bufs=2, name="pbm")
                nc.tensor.matmul(pbm, ones1, negmx, start=True, stop=True)
                negm = scratch.tile([128, 1], fp32, tag="negm")
                nc.scalar.copy(negm, pbm)
                a_b = persist.tile([128, TS, QT], fp32, tag=f"a{b}")
                nc.scalar.activation(a_b, eij, AF.Exp, bias=negm, scale=1.0)
                nc.vector.tensor_tensor(a_b, a_b, mask_f[b], op=OP.mult)

                s1 = scratch.tile([128, 1], fp32, tag="s1")
                nc.vector.reduce_sum(s1, a_b, axis=mybir.AxisListType.XY)
                ssum = pp.tile([1, 1], fp32, tag="pb", bufs=2, name="ssum")
                nc.tensor.matmul(ssum, s1, ones_col, start=True, stop=True)
                den = scratch.tile([1, 1], fp32, tag="den")
                nc.vector.tensor_scalar_add(den, ssum, EPS)
                rden = scratch.tile([1, 1], fp32, tag="rden")
                nc.vector.reciprocal(rden, den)

                # bf16 copy of the softmax weights for the PE pooling
                a_bf = persist.tile([128, TS, QT], bf16, tag=f"abf{b}")
                nc.scalar.copy(a_bf, a_b)

                # out[b, d] = rden * sum_s a[s] x[s, d]   (bf16 PE pooling)
                orow = persist.tile([1, D], fp32, tag="rowx", bufs=2, name="orow")
                for h in range(2):
                    po = pp.tile([1, 512], fp32, tag="pb", bufs=2, name="po")
                    n = 0
                    for t in range(TS):
                        for q in range(QT):
                            nc.tensor.matmul(
                                po,
                                a_bf[:, t, q : q + 1],
                                xbf_tiles[b][t][
                                    :, q * D + h * 512 : q * D + (h + 1) * 512
                                ],
                                start=(n == 0),
                                stop=(n == TS * QT - 1),
                            )
                            n += 1
                    nc.vector.tensor_scalar_mul(
                        orow[:, ts(h, 512)], po, rden
                    )
                out_rows.append(orow)
                nc.sync.dma_start(out=out_d[b : b + 1, :], in_=orow)

    nc.compile()
    return nc


def _get_nc():
    if "nc" not in _CACHE:
        _CACHE["nc"] = _build()
    return _CACHE["nc"]


def run(inputs, trace=False):
    from concourse.bass_utils import run_bass_kernel_spmd

    x = np.ascontiguousarray(inputs["x"], dtype=np.float32)
    mask = np.ascontiguousarray(inputs["mask"], dtype=np.int32)
    c = np.ascontiguousarray(inputs["c"], dtype=np.float32)
    W = np.ascontiguousarray(inputs["W"], dtype=np.float32)
    Wc = np.ascontiguousarray(inputs["Wc"], dtype=np.float32)
    b = np.ascontiguousarray(inputs["b"], dtype=np.float32)
    scale = np.ascontiguousarray(inputs["scale"], dtype=np.float32)

    in_maps = []
    for i in range(NCORES):
        sl = slice(i * BPC, (i + 1) * BPC)
        in_maps.append(
            {
                "x": x[sl],
                "mask": mask[sl],
                "c": c[sl],
                "W": W,
                "Wc": Wc,
                "b": b,
                "scale": scale,
            }
        )

    nc = _get_nc()
    res = run_bass_kernel_spmd(
        nc, in_maps, core_ids=list(range(NCORES)), trace=trace
    )
    out = np.concatenate([res.results[i]["out"] for i in range(NCORES)], axis=0)
    return out.astype(np.float32), res


def kernel(**inputs):
    out, _ = run(inputs, trace=False)
    return out
